# revision 15
# baseline (speedup 1.0000x reference)
"""Causal multi-head attention block (qkv proj + attention + out proj) on 8
Trainium2 NeuronCores.

Sharding: Megatron-style tensor parallel over heads — 2 heads per core.
Each core computes its heads' Q/K/V projections (column-sharded w_qkv),
causal attention for those heads, and a row-sharded partial of the output
projection.  The host sums the 8 partial outputs and adds b_o.

Device-side layout notes:
 - The host feeds X^T [C, B*T] (bf16) so every matmul contraction dim (C,
   head dim, or key position) lands on SBUF partitions with no on-device
   transposes of activations.  Scores are computed transposed
   (S^T[k, q] = K^T.T @ Q^T per 128-wide k block), softmax-exp runs on the
   scalar engine, and the denominator is produced by an extra all-ones
   column appended to V (row 64 of the attn@V accumulator).
 - Matmuls run in bf16 (fp32 PSUM accumulation).  The V^T->V transpose
   runs through the PE transpose path in fp32r.
 - Softmax reciprocal = exp(-ln(x)) on the scalar engine (the DVE exact
   reciprocal is ~6.4ns/element and too slow; reciprocal_approx_fast is
   broken on this image).
"""

import numpy as np
import ml_dtypes

import concourse.bass as bass
import concourse.tile as tile
import concourse.mybir as mybir
from concourse import bacc
from concourse.bass_utils import run_bass_kernel_spmd

B, T, C, H, DH = 4, 2048, 1024, 16, 64
NCORES = 8
HPC = H // NCORES            # heads per core = 2
R = B * T                    # 8192 rows
HD = HPC * DH                # 128 local head dims
KT = C // 128                # 8 contraction tiles over C
RC = 512                     # row chunk in qkv stage
QC = 512                     # query chunk in attention
NQC = T // QC                # 4
NKB = T // 128               # 16 key blocks

F32 = mybir.dt.float32
F32R = mybir.dt.float32r
BF16 = mybir.dt.bfloat16

LAST_RESULT = None           # BassKernelResults of the most recent run
_CACHED_NC = None
DEBUG = False


def _emit(nc, tc, xt, wqkv, bqkv, wo, tri, ident, y, dbg=None):
    from contextlib import ExitStack

    Exp = mybir.ActivationFunctionType.Exp
    Ln = mybir.ActivationFunctionType.Ln
    with ExitStack() as ctx:
        const = ctx.enter_context(tc.tile_pool(name="const", bufs=1))
        bigp = ctx.enter_context(tc.tile_pool(name="bigp", bufs=2))
        xtp = ctx.enter_context(tc.tile_pool(name="xtp", bufs=3))
        vtp = ctx.enter_context(tc.tile_pool(name="vtp", bufs=2))
        vsbp = ctx.enter_context(tc.tile_pool(name="vsbp", bufs=2))
        ptp = ctx.enter_context(tc.tile_pool(name="ptp", bufs=3))
        osbp = ctx.enter_context(tc.tile_pool(name="osbp", bufs=2))
        ystp = ctx.enter_context(tc.tile_pool(name="ystp", bufs=4))
        smallp = ctx.enter_context(tc.tile_pool(name="smallp", bufs=2))
        psA = ctx.enter_context(tc.tile_pool(name="psA", bufs=2, space="PSUM"))
        psS = ctx.enter_context(tc.tile_pool(name="psS", bufs=1, space="PSUM"))
        psO = ctx.enter_context(tc.tile_pool(name="psO", bufs=2, space="PSUM"))

        # ---- constants ----
        w_sb = const.tile([128, KT, 3 * HD], BF16, name="w_sb")
        wqkv_r = wqkv[:, :].rearrange("(ko ki) m -> ki ko m", ki=128)
        nc.sync.dma_start(out=w_sb[:, :, :], in_=wqkv_r[:, :, :])
        wo_sb = const.tile([128, C], BF16, name="wo_sb")
        nc.sync.dma_start(out=wo_sb[:, :], in_=wo[:, :])
        b_sb = const.tile([128, 3], F32, name="b_sb")
        for m in range(3):
            nc.sync.dma_start(
                out=b_sb[:, m : m + 1],
                in_=bqkv[m : m + 1, :].rearrange("a n -> n a"),
            )
        tri_sb = const.tile([128, 128], BF16, name="tri_sb")
        nc.sync.dma_start(out=tri_sb[:, :], in_=tri[:, :])
        id_sb = const.tile([128, 128], F32R, name="id_sb")
        nc.sync.dma_start(out=id_sb[:, :], in_=ident[:, :])

        xt_r = xt[:, :].rearrange("(ko ki) n -> ki ko n", ki=128)

        for b in range(B):
            obase = b * T
            # Q^T / K^T for this batch: [dim(2 heads x 64), row-in-batch]
            qt = bigp.tile([128, T], BF16, name="qt", tag="qt")
            ktt = bigp.tile([128, T], BF16, name="ktt", tag="ktt")

            # ---- stage A: qkv projection for this batch's rows ----
            vt_b = vtp.tile([128, T], F32R, name="vt_b", tag="vt")
            for rcl in range(T // RC):
                rc = b * (T // RC) + rcl
                x_t = xtp.tile([128, KT, RC], BF16, name="x_t", tag="xt")
                nc.sync.dma_start(
                    out=x_t[:, :, :], in_=xt_r[:, :, rc * RC : (rc + 1) * RC]
                )
                for m in range(3):
                    ps = psA.tile([128, RC], F32, name="ps_qkv", tag="qkv")
                    for k in range(KT):
                        nc.tensor.matmul(
                            ps[:, :],
                            lhsT=w_sb[:, k, m * HD : (m + 1) * HD],
                            rhs=x_t[:, k, :],
                            start=(k == 0),
                            stop=(k == KT - 1),
                        )
                    if m == 0:
                        dst = qt[:, rcl * RC : (rcl + 1) * RC]
                    elif m == 1:
                        dst = ktt[:, rcl * RC : (rcl + 1) * RC]
                    else:
                        dst = vt_b[:, rcl * RC : (rcl + 1) * RC]
                    nc.vector.tensor_scalar_add(
                        out=dst, in0=ps[:, :], scalar1=b_sb[:, m : m + 1]
                    )

            if dbg is not None and b == 0:
                nc.sync.dma_start(out=dbg["qt"][:, 0:T], in_=qt[:, :])
                nc.sync.dma_start(out=dbg["ktt"][:, 0:T], in_=ktt[:, :])

            # ---- V^T -> V_aug (PE transpose), cols: [V_h0 | 1 | V_h1 | 1] ----
            v_sb = vsbp.tile([128, NKB, 130], BF16, name="v_sb", tag="vsb")
            # ones columns for the softmax-denominator rows (tri col 127 == 1s)
            ones_src = tri[:, 127:128].unsqueeze(1).broadcast_to([128, NKB, 1])
            nc.sync.dma_start(out=v_sb[:, :, 64:65], in_=ones_src)
            nc.sync.dma_start(out=v_sb[:, :, 129:130], in_=ones_src)
            for kb in range(NKB):
                tps = psA.tile([128, 128], F32R, name="tps", tag="qkv")
                nc.tensor.transpose(
                    tps[:, :], vt_b[:, kb * 128 : (kb + 1) * 128], id_sb[:, :]
                )
                nc.vector.tensor_copy(
                    out=v_sb[:, kb, 0:64], in_=tps[:, 0:64].bitcast(F32)
                )
                nc.vector.tensor_copy(
                    out=v_sb[:, kb, 65:129], in_=tps[:, 64:128].bitcast(F32)
                )

            # ---- attention ----
            o_sb = osbp.tile([128, T], BF16, name="o_sb", tag="osb")
            for qc in range(NQC):
                o_ps = [
                    psO.tile([65, QC], F32, name=f"o_ps{h}", tag=f"o{h}")
                    for h in range(2)
                ]
                nkb = 4 * qc + 4
                for kb in range(nkb):
                    off = max(0, (kb - 4 * qc) * 128)
                    n = QC - off
                    s_ps = [
                        psS.tile([128, QC], F32, name=f"s_ps{h}", tag=f"s{h}")
                        for h in range(2)
                    ]
                    for h in range(2):
                        nc.tensor.matmul(
                            s_ps[h][:, 0:n],
                            lhsT=ktt[
                                64 * h : 64 * h + 64,
                                kb * 128 : (kb + 1) * 128,
                            ],
                            rhs=qt[
                                64 * h : 64 * h + 64,
                                qc * QC + off : (qc + 1) * QC,
                            ],
                            start=True,
                            stop=True,
                        )
                    p_t = [
                        ptp.tile([128, QC], BF16, name=f"p_t{h}", tag=f"pt{h}")
                        for h in range(2)
                    ]
                    for h in range(2):
                        nc.scalar.activation(
                            out=p_t[h][:, 0:n], in_=s_ps[h][:, 0:n], func=Exp
                        )
                        if kb >= 4 * qc:
                            # diagonal block: upper-tri (q >= k) keep mask
                            nc.gpsimd.tensor_mul(
                                out=p_t[h][:, 0:128],
                                in0=p_t[h][:, 0:128],
                                in1=tri_sb[:, :],
                            )
                    if dbg is not None and b == 0 and qc == 0 and kb == 0:
                        for h in range(2):
                            nc.sync.dma_start(
                                out=dbg["pt0"][:, h, :], in_=p_t[h][:, :]
                            )
                    for h in range(2):
                        nc.tensor.matmul(
                            o_ps[h][:, off:QC],
                            lhsT=v_sb[:, kb, 65 * h : 65 * h + 65],
                            rhs=p_t[h][:, 0:n],
                            start=(kb == 0),
                            stop=(kb == nkb - 1),
                            skip_group_check=True,
                        )
                if dbg is not None and b == 0 and qc == 0:
                    for h in range(2):
                        dstage = smallp.tile(
                            [65, QC], F32, name="dstage", tag="dstage"
                        )
                        nc.vector.tensor_copy(out=dstage[:, :], in_=o_ps[h][:, :])
                        nc.sync.dma_start(out=dbg[f"onum{h}"][:, :], in_=dstage[:, :])
                # softmax denominators live in row 64 of each o_ps
                for h in range(2):
                    srow = smallp.tile([65, QC], F32, name="srow", tag=f"srow{h}")
                    srln = smallp.tile([65, QC], F32, name="srln", tag=f"srln{h}")
                    nc.vector.tensor_copy(out=srow[64:65, :], in_=o_ps[h][64:65, :])
                    # 1/x = exp(-ln(x)) on the scalar engine
                    nc.scalar.activation(
                        out=srln[64:65, :], in_=srow[64:65, :], func=Ln
                    )
                    nc.scalar.activation(
                        out=srow[64:65, :], in_=srln[64:65, :], func=Exp, scale=-1.0
                    )
                    srow0 = smallp.tile([1, QC], F32, name="srow0", tag=f"sr0{h}")
                    nc.sync.dma_start(out=srow0[0:1, :], in_=srow[64:65, :])
                    bch = smallp.tile([64, QC], F32, name="bch", tag=f"bc{h}")
                    nc.gpsimd.partition_broadcast(
                        out_ap=bch[:, :], in_ap=srow0[0:1, :]
                    )
                    if dbg is not None and b == 0 and qc == 0:
                        nc.sync.dma_start(
                            out=dbg["srow"][h : h + 1, :], in_=srow[64:65, :]
                        )
                        nc.sync.dma_start(out=dbg["bch"][h, :, :], in_=bch[:, :])
                    if h == 0:
                        nc.vector.tensor_mul(
                            out=o_sb[0:64, qc * QC : (qc + 1) * QC],
                            in0=o_ps[0][0:64, :],
                            in1=bch[:, :],
                        )
                    else:
                        htmp = smallp.tile([64, QC], BF16, name="htmp", tag="htmp")
                        nc.vector.tensor_mul(
                            out=htmp[:, :], in0=o_ps[1][0:64, :], in1=bch[:, :]
                        )
                        # lane shift h1 dims to partitions 64:128 (DMA repartitions)
                        nc.sync.dma_start(
                            out=o_sb[64:128, qc * QC : (qc + 1) * QC], in_=htmp[:, :]
                        )

                # ---- output projection for this qc's rows ----
                for rb in range(qc * 4, qc * 4 + 4):
                    for j in range(C // 512):
                        yps = psA.tile([128, 512], F32, name="yps", tag="qkv")
                        nc.tensor.matmul(
                            yps[:, :],
                            lhsT=o_sb[:, rb * 128 : (rb + 1) * 128],
                            rhs=wo_sb[:, j * 512 : (j + 1) * 512],
                            start=True,
                            stop=True,
                        )
                        yst = ystp.tile([128, 512], F32, name="yst", tag="yst")
                        nc.vector.tensor_copy(out=yst[:, :], in_=yps[:, :])
                        nc.sync.dma_start(
                            out=y[
                                obase + rb * 128 : obase + (rb + 1) * 128,
                                j * 512 : (j + 1) * 512,
                            ],
                            in_=yst[:, :],
                        )
            if dbg is not None and b == 0:
                nc.sync.dma_start(out=dbg["vsb"][:, :], in_=v_sb[:, :, :])
                nc.sync.dma_start(out=dbg["osb"][:, :], in_=o_sb[:, :])


def _build():
    nc = bacc.Bacc("TRN2", target_bir_lowering=False)
    xt = nc.dram_tensor("xt", [C, R], BF16, kind="ExternalInput")
    wqkv = nc.dram_tensor("wqkv", [C, 3 * HD], BF16, kind="ExternalInput")
    bqkv = nc.dram_tensor("bqkv", [3, HD], F32, kind="ExternalInput")
    wo = nc.dram_tensor("wo", [HD, C], BF16, kind="ExternalInput")
    tri = nc.dram_tensor("tri", [128, 128], BF16, kind="ExternalInput")
    ident = nc.dram_tensor("ident", [128, 128], F32R, kind="ExternalInput")
    y = nc.dram_tensor("y", [R, C], F32, kind="ExternalOutput")
    dbg = None
    if DEBUG:
        dbg = {
            "qt": nc.dram_tensor("d_qt", [128, R], BF16, kind="ExternalOutput"),
            "ktt": nc.dram_tensor("d_ktt", [128, R], BF16, kind="ExternalOutput"),
            "vsb": nc.dram_tensor(
                "d_vsb", [128, NKB * 130], BF16, kind="ExternalOutput"
            ),
            "osb": nc.dram_tensor("d_osb", [128, T], BF16, kind="ExternalOutput"),
            "onum0": nc.dram_tensor("d_onum0", [65, QC], F32, kind="ExternalOutput"),
            "onum1": nc.dram_tensor("d_onum1", [65, QC], F32, kind="ExternalOutput"),
            "srow": nc.dram_tensor("d_srow", [2, QC], F32, kind="ExternalOutput"),
            "bch": nc.dram_tensor("d_bch", [2, 64, QC], F32, kind="ExternalOutput"),
            "pt0": nc.dram_tensor(
                "d_pt0", [128, 2, QC], BF16, kind="ExternalOutput"
            ),
        }
    with tile.TileContext(nc) as tc:
        _emit(nc, tc, xt, wqkv, bqkv, wo, tri, ident, y, dbg)
    nc.finalize()
    return nc


def kernel(hidden_states, w_qkv, b_qkv, w_o, b_o):
    global LAST_RESULT, _CACHED_NC
    X = np.ascontiguousarray(np.asarray(hidden_states, dtype=np.float32)).reshape(
        R, C
    )
    w_qkv = np.asarray(w_qkv, dtype=np.float32)
    b_qkv = np.asarray(b_qkv, dtype=np.float32)
    w_o = np.asarray(w_o, dtype=np.float32)
    b_o = np.asarray(b_o, dtype=np.float32)

    bf = ml_dtypes.bfloat16
    Xt = np.ascontiguousarray(X.T).astype(bf)  # [C, R]
    scale = float(DH) ** -0.5
    tri_m = np.triu(np.ones((128, 128), dtype=np.float32)).astype(bf)
    ident = np.eye(128, dtype=np.float32)

    in_maps = []
    for c in range(NCORES):
        heads = [HPC * c + i for i in range(HPC)]
        wcols, bcols = [], []
        for sec in range(3):  # q, k, v
            sc = scale if sec == 0 else 1.0
            for h in heads:
                lo = sec * C + h * DH
                wcols.append(w_qkv[:, lo : lo + DH] * sc)
                bcols.append(b_qkv[lo : lo + DH] * sc)
        wqkv_c = np.ascontiguousarray(np.concatenate(wcols, axis=1)).astype(bf)
        bqkv_c = np.ascontiguousarray(np.concatenate(bcols).reshape(3, HD))
        wo_c = np.ascontiguousarray(
            np.concatenate([w_o[h * DH : (h + 1) * DH, :] for h in heads], axis=0)
        ).astype(bf)  # [HD, C]
        in_maps.append(
            {
                "xt": Xt,
                "wqkv": wqkv_c,
                "bqkv": bqkv_c,
                "wo": wo_c,
                "tri": tri_m,
                "ident": ident,
            }
        )

    if _CACHED_NC is None:
        _CACHED_NC = _build()
    res = run_bass_kernel_spmd(_CACHED_NC, in_maps, core_ids=list(range(NCORES)))
    LAST_RESULT = res

    out = res.results[0]["y"].astype(np.float64)
    for c in range(1, NCORES):
        out += res.results[c]["y"]
    out += b_o
    return out.astype(np.float32).reshape(B, T, C)
